# revision 16
# baseline (speedup 1.0000x reference)
"""AnchorLoss Trainium2 kernel.

loss = sum_{b,i,j: mask[b,i,j]==1} (1 - exp(-|z_i - z_j|^2 / 10)),  z = embedding + abs_coords

Sharding: data-parallel over batch B=8, one batch per NeuronCore. Each core:
  - device-side prep: z = e + a, r = |z|^2, bf16 hi/lo splits (pseudo-fp32),
  - streams its [2048, 2048] bf16 mask (host-cast from {0,1} int32, exact)
    in 16 row-blocks of [128, 2048],
  - per 512-col chunk: K=14 bf16 matmul -> PSUM = d2, with chunks
    alternating between PE sub-tiles T0/T8 (32x128 row-tiled mode, two
    copies of the small operands at partitions 0 and 64) so matmuls
    overlap; one ScalarE exp per block (scale=-0.1, PSUM -> SBUF bf16);
    one wide VectorE (E - 1) * mask with per-partition accumulate,
  - reduces the accumulator columns to a single scalar on device,
  - returns [1, 1] partials; host sums 8 scalars and negates.

The host passes e/a stacked+transposed+folded as one [16, N/4] array
(layout only, zero flops): row d*8+g holds [e_d chunk g | a_d chunk g],
so prep ops run 16-partition-wide and the coordinate load is one small
DMA. The mask cast int32 {0,1} -> bf16 is exact and halves the HBM
stream, which is the dominant traffic.
"""
import numpy as np
import sys

for _p in ("/opt/trn_rl_repo", "/root/.axon_site/_ro/trn_rl_repo"):
    if _p not in sys.path:
        sys.path.append(_p)

N = 2048
B = 8

_CACHED = None


def _build(n=N):
    from concourse import bacc, mybir, tile
    from concourse.tile import add_dep_helper

    f32 = mybir.dt.float32
    bf16 = mybir.dt.bfloat16
    AF = mybir.ActivationFunctionType
    ALU = mybir.AluOpType

    nb = n // 128          # mask row blocks

    G = 64                 # prep fold factor (2G = 128: full-width prep ops)
    w = n // G             # folded chunk width
    nc = bacc.Bacc()
    ea_in = nc.declare_dram_parameter("ea", [2 * G, 2 * w], f32, isOutput=False)
    m_in = nc.declare_dram_parameter("m", [n, n], bf16, isOutput=False)
    out = nc.declare_dram_parameter("out", [1, 1], f32, isOutput=True)

    with tile.TileContext(nc) as tc:
        with (
            tc.tile_pool(name="singles", bufs=1) as singles,
            tc.tile_pool(name="maskp", bufs=8) as maskp,
            tc.tile_pool(name="expp", bufs=4) as expp,
            tc.tile_pool(name="psum", bufs=2, space="PSUM") as psump,
        ):
            # ---- coordinate load first: it heads the critical path ----
            ea = singles.tile([2 * G, 2 * w], f32)  # [e_d chunk g | a_d chunk g]
            nc.sync.dma_start(ea[:], ea_in[:])

            # warm the ACT exp table set off the critical path
            dummy = singles.tile([1, 8], f32)
            nc.gpsimd.memset(dummy[:], 0.0)
            nc.scalar.activation(dummy[:], dummy[:], AF.Exp)

            # K=14 row pairing (lhsT row k x rhs row k) -> PSUM = d2
            # (r_i + r_j enter as their four z^2 hi/lo components):
            #  k0-3:  1_i * [sqxh, sqyh, sqxl, sqyl]_j
            #  k4-7:  [sqxh, sqyh, sqxl, sqyl]_i * 1_j
            #  k8:  zxh*m2zxh  k9:  zyh*m2zyh  k10: zxh*m2zxl
            #  k11: zyh*m2zyl  k12: zxl*m2zxh  k13: zyl*m2zyh
            # Placement DMAs linearize folded [2G, w] (partition-major)
            # into [2, n] rows (free-major) -- same element order.
            # Issues balanced over the three DMA-capable queues; a
            # second copy at partitions 64:78 feeds PE sub-tile T8
            # (32x128 row-tiled mode) so consecutive chunks' matmuls
            # execute concurrently.
            zcol = singles.tile([78, n], bf16)  # rhs (j side)
            zrow = singles.tile([78, n], bf16)  # lhsT (i side)

            # constant rows first: no data deps, so these placements
            # (and the first-DMA overhead of their queues) land during
            # the prep compute
            ones4 = singles.tile([4, n], bf16)
            nc.vector.memset(ones4[:], 1.0)
            nc.gpsimd.dma_start(zcol[4:8, :], ones4[:])
            nc.scalar.dma_start(zrow[0:4, :], ones4[:])

            # ---- prep (folded [2G, w] layout; row d*G+g = coord d, chunk g) ----
            zt = singles.tile([2 * G, w], f32)
            nc.vector.tensor_tensor(zt[:], ea[:, 0:w], ea[:, w:2 * w], ALU.add)
            sq = singles.tile([2 * G, w], f32)
            nc.vector.tensor_tensor(sq[:], zt[:], zt[:], ALU.mult)

            # ---- bf16 hi/lo splits: hi = bf16(v), lo = bf16(v - hi) ----
            zh = singles.tile([2 * G, w], bf16)
            zl = singles.tile([2 * G, w], bf16)
            sqh = singles.tile([2 * G, w], bf16)
            sql = singles.tile([2 * G, w], bf16)
            m2zh = singles.tile([2 * G, w], bf16)   # -2 * zh (exact in bf16)
            m2zl = singles.tile([2 * G, w], bf16)
            nc.scalar.activation(zh[:], zt[:], AF.Copy)
            nc.vector.tensor_tensor(zl[:], zt[:], zh[:], ALU.subtract)
            nc.scalar.activation(sqh[:], sq[:], AF.Copy)
            nc.vector.tensor_tensor(sql[:], sq[:], sqh[:], ALU.subtract)
            nc.vector.tensor_scalar_mul(m2zh[:], zh[:], -2.0)
            nc.vector.tensor_scalar_mul(m2zl[:], zl[:], -2.0)

            nc.gpsimd.dma_start(zcol[0:2, :], sqh[:])
            nc.sync.dma_start(zcol[8:10, :], m2zh[:])
            nc.scalar.dma_start(zcol[2:4, :], sql[:])
            nc.gpsimd.dma_start(zcol[10:12, :], m2zl[:])
            zcol_p = nc.sync.dma_start(zcol[12:14, :], m2zh[:])

            nc.sync.dma_start(zrow[4:6, :], sqh[:])
            nc.gpsimd.dma_start(zrow[8:10, :], zh[:])
            nc.scalar.dma_start(zrow[6:8, :], sql[:])
            nc.gpsimd.dma_start(zrow[10:12, :], zh[:])
            zrow_p = nc.sync.dma_start(zrow[12:14, :], zl[:])

            zcol_tail = nc.gpsimd.dma_start(zcol[64:78, :], zcol[0:14, :])
            zrow_tail = nc.scalar.dma_start(zrow[64:78, :], zrow[0:14, :])

            acc = singles.tile([128, nb + 4], f32)

            # ---- main loop: nb row blocks ----
            acol = 0
            for ib in range(nb):
                mk = maskp.tile([128, n], bf16)
                # keep the SDMA engines clear for the prep placement
                # DMAs (the mask stream hides under the STT pace):
                # block 0 rides the scalar queue so its completion
                # semaphore doesn't rotate into the placement waits on
                # sync; later blocks wait for the T8 replication copies
                if ib == 0:
                    mdma = nc.scalar.dma_start(mk[:], m_in[0:128, :])
                    add_dep_helper(mdma.ins, zcol_p.ins,
                                   reason="defer mask stream behind prep")
                    add_dep_helper(mdma.ins, zrow_p.ins,
                                   reason="defer mask stream behind prep")
                else:
                    mdma = nc.sync.dma_start(mk[:],
                                             m_in[ib * 128:(ib + 1) * 128, :])
                    add_dep_helper(mdma.ins, zcol_tail.ins,
                                   reason="defer mask stream behind prep")
                    add_dep_helper(mdma.ins, zrow_tail.ins,
                                   reason="defer mask stream behind prep")
                # exp results for a whole row block
                eb = expp.tile([128, n], bf16)
                # chunking: fine at the start to fill the pipeline fast
                if ib == 0:
                    awidths, swidths = [512] * 4, [512] * 4
                elif ib == 1:
                    awidths, swidths = [1024] * 2, [1024] * 2
                else:
                    awidths, swidths = [2048], [2048]
                ps = psump.tile([128, n], f32)
                col = 0
                for wch in awidths:
                    for jc in range(wch // 512):
                        c0 = col + jc * 512
                        g = 64 * ((c0 // 512) % 2)  # PE sub-tile T0 / T8
                        nc.tensor.matmul(
                            ps[:, c0:c0 + 512],
                            zrow[g:g + 14, ib * 128:(ib + 1) * 128],
                            zcol[g:g + 14, c0:c0 + 512],
                            start=True,
                            stop=True,
                        )
                    nc.scalar.activation(eb[:, col:col + wch],
                                         ps[:, col:col + wch], AF.Exp,
                                         scale=-0.1)
                    col += wch
                col = 0
                for wch in swidths:
                    nc.vector.scalar_tensor_tensor(
                        eb[:, col:col + wch], eb[:, col:col + wch], 1.0,
                        mk[:, col:col + wch],
                        op0=ALU.subtract, op1=ALU.mult,
                        accum_out=acc[:, acol:acol + 1],
                    )
                    col += wch
                    acol += 1
            # [128, acol] partials -> one on-device scalar (a [1, 1] DMA
            # needs a single descriptor; [128, 1] needs 128)
            accr = singles.tile([128, 1], f32)
            nc.vector.tensor_reduce(accr[:], acc[:, 0:acol],
                                    mybir.AxisListType.X, ALU.add)
            accs = singles.tile([1, 1], f32)
            nc.gpsimd.tensor_reduce(accs[:], accr[:],
                                    mybir.AxisListType.C, ALU.add)
            nc.scalar.dma_start(out[:], accs[:])
    nc.compile()
    return nc


def _get_graph():
    global _CACHED
    if _CACHED is None:
        _CACHED = _build()
    return _CACHED


def _pack_ea(e, a, n, G=64):
    w = n // G
    ea = np.empty((2 * G, 2 * w), dtype=np.float32)
    for d in range(2):
        ea[d * G:(d + 1) * G, :w] = e[:, d].reshape(G, w)
        ea[d * G:(d + 1) * G, w:] = a[:, d].reshape(G, w)
    return ea


def kernel(embedding, abs_coords, patch_mask, _trace=False, _trace_kwargs=None):
    import ml_dtypes
    from concourse.bass_utils import run_bass_kernel_spmd

    nc = _get_graph()
    mask_bf16 = np.ascontiguousarray(patch_mask).astype(ml_dtypes.bfloat16)
    in_maps = [
        {
            "ea": _pack_ea(embedding[b], abs_coords[b], N),
            "m": mask_bf16[b],
        }
        for b in range(B)
    ]
    kw = {}
    if _trace:
        kw = dict(trace=True, **(_trace_kwargs or {}))
    res = None
    last_err = None
    for _attempt in range(3):
        try:
            res = run_bass_kernel_spmd(nc, in_maps, core_ids=list(range(B)), **kw)
            # force materialization so device-side failures surface here
            total = -sum(
                float(np.sum(np.asarray(r["out"]), dtype=np.float64))
                for r in res.results
            )
            break
        except Exception as err:  # transient device faults: retry
            last_err = err
            res = None
    if res is None:
        raise last_err
    out = np.float32(total)
    if _trace:
        return out, res
    return out


# revision 19
# speedup vs baseline: 1.0563x; 1.0563x over previous
"""AnchorLoss Trainium2 kernel.

loss = sum_{b,i,j: mask[b,i,j]==1} (1 - exp(-|z_i - z_j|^2 / 10)),  z = embedding + abs_coords

Sharding: data-parallel over batch B=8, one batch per NeuronCore. Each core:
  - device-side prep: z = e + a, r = |z|^2, bf16 hi/lo splits (pseudo-fp32),
  - streams its [2048, 2048] bf16 mask (host-cast from {0,1} int32, exact)
    in 16 row-blocks of [128, 2048],
  - per 512-col chunk: K=14 bf16 matmul -> PSUM = d2, with chunks
    alternating between PE sub-tiles T0/T8 (32x128 row-tiled mode, two
    copies of the small operands at partitions 0 and 64) so matmuls
    overlap; one ScalarE exp per block (scale=-0.1, PSUM -> SBUF bf16);
    one wide VectorE (E - 1) * mask with per-partition accumulate,
  - reduces the accumulator columns to a single scalar on device,
  - returns [1, 1] partials; host sums 8 scalars and negates.

The host passes e/a stacked+transposed+folded as one [16, N/4] array
(layout only, zero flops): row d*8+g holds [e_d chunk g | a_d chunk g],
so prep ops run 16-partition-wide and the coordinate load is one small
DMA. The mask cast int32 {0,1} -> bf16 is exact and halves the HBM
stream, which is the dominant traffic.
"""
import numpy as np
import sys

for _p in ("/opt/trn_rl_repo", "/root/.axon_site/_ro/trn_rl_repo"):
    if _p not in sys.path:
        sys.path.append(_p)

N = 2048
B = 8

_CACHED = None


def _build(n=N):
    from concourse import bacc, mybir, tile
    from concourse.tile import add_dep_helper

    f32 = mybir.dt.float32
    bf16 = mybir.dt.bfloat16
    AF = mybir.ActivationFunctionType
    ALU = mybir.AluOpType

    nb = n // 128          # mask row blocks

    G = 8                  # prep fold factor
    w = n // G             # folded chunk width
    nc = bacc.Bacc()
    ea_in = nc.declare_dram_parameter("ea", [2 * G, 2 * w], f32, isOutput=False)
    m_in = nc.declare_dram_parameter("m", [n, n], bf16, isOutput=False)
    out = nc.declare_dram_parameter("out", [1, 1], f32, isOutput=True)

    with tile.TileContext(nc) as tc:
        with (
            tc.tile_pool(name="singles", bufs=1) as singles,
            tc.tile_pool(name="maskp", bufs=8) as maskp,
            tc.tile_pool(name="expp", bufs=4) as expp,
            tc.tile_pool(name="psum", bufs=2, space="PSUM") as psump,
        ):
            # ---- coordinate load first: it heads the critical path ----
            ea = singles.tile([2 * G, 2 * w], f32)  # [e_d chunk g | a_d chunk g]
            nc.sync.dma_start(ea[:], ea_in[:])

            # warm the ACT exp table set off the critical path
            dummy = singles.tile([1, 8], f32)
            nc.gpsimd.memset(dummy[:], 0.0)
            nc.scalar.activation(dummy[:], dummy[:], AF.Exp)

            # K=14 row pairing (lhsT row k x rhs row k) -> PSUM = d2
            # (r_i + r_j enter as their four z^2 hi/lo components):
            #  k0-3:  1_i * [sqxh, sqyh, sqxl, sqyl]_j
            #  k4-7:  [sqxh, sqyh, sqxl, sqyl]_i * 1_j
            #  k8:  zxh*m2zxh  k9:  zyh*m2zyh  k10: zxh*m2zxl
            #  k11: zyh*m2zyl  k12: zxl*m2zxh  k13: zyl*m2zyh
            # Placement DMAs linearize folded [2G, w] (partition-major)
            # into [2, n] rows (free-major) -- same element order.
            # Issues balanced over the three DMA-capable queues; a
            # second copy at partitions 64:78 feeds PE sub-tile T8
            # (32x128 row-tiled mode) so consecutive chunks' matmuls
            # execute concurrently.
            zcol = singles.tile([78, n], bf16)  # rhs (j side)
            zrow = singles.tile([78, n], bf16)  # lhsT (i side)

            # constant rows first: no data deps, so these placements
            # (and the first-DMA overhead of their queues) land during
            # the prep compute
            ones4 = singles.tile([4, n], bf16)
            nc.gpsimd.memset(ones4[:], 1.0)
            nc.gpsimd.dma_start(zcol[4:8, :], ones4[:])
            nc.scalar.dma_start(zrow[0:4, :], ones4[:])

            # ---- prep (folded [2G, w] layout; row d*G+g = coord d, chunk g) ----
            zt = singles.tile([2 * G, w], f32)
            nc.vector.tensor_tensor(zt[:], ea[:, 0:w], ea[:, w:2 * w], ALU.add)
            sq = singles.tile([2 * G, w], f32)
            nc.vector.tensor_tensor(sq[:], zt[:], zt[:], ALU.mult)

            # ---- bf16 hi/lo splits: hi = bf16(v), lo = bf16(v - hi) ----
            zh = singles.tile([2 * G, w], bf16)
            zl = singles.tile([2 * G, w], bf16)
            sqh = singles.tile([2 * G, w], bf16)
            sql = singles.tile([2 * G, w], bf16)
            m2zh = singles.tile([2 * G, w], bf16)   # -2 * zh (exact in bf16)
            m2zl = singles.tile([2 * G, w], bf16)
            nc.scalar.activation(zh[:], zt[:], AF.Copy)
            nc.vector.tensor_tensor(zl[:], zt[:], zh[:], ALU.subtract)
            nc.scalar.activation(sqh[:], sq[:], AF.Copy)
            nc.vector.tensor_tensor(sql[:], sq[:], sqh[:], ALU.subtract)
            nc.vector.tensor_scalar_mul(m2zh[:], zh[:], -2.0)
            nc.vector.tensor_scalar_mul(m2zl[:], zl[:], -2.0)

            nc.gpsimd.dma_start(zcol[0:2, :], sqh[:])
            nc.sync.dma_start(zcol[8:10, :], m2zh[:])
            nc.scalar.dma_start(zcol[2:4, :], sql[:])
            nc.gpsimd.dma_start(zcol[10:12, :], m2zl[:])
            zcol_p = nc.sync.dma_start(zcol[12:14, :], m2zh[:])

            nc.sync.dma_start(zrow[4:6, :], sqh[:])
            nc.gpsimd.dma_start(zrow[8:10, :], zh[:])
            nc.scalar.dma_start(zrow[6:8, :], sql[:])
            nc.gpsimd.dma_start(zrow[10:12, :], zh[:])
            zrow_p = nc.sync.dma_start(zrow[12:14, :], zl[:])

            zcol_tail = nc.gpsimd.dma_start(zcol[64:78, :], zcol[0:14, :])
            zrow_tail = nc.scalar.dma_start(zrow[64:78, :], zrow[0:14, :])

            acc = singles.tile([128, nb + 4], f32)

            # ---- main loop: nb row blocks ----
            acol = 0
            for ib in range(nb):
                mk = maskp.tile([128, n], bf16)
                # keep the SDMA engines clear for the prep placement
                # DMAs (the mask stream hides under the STT pace):
                # block 0 rides the scalar queue so its completion
                # semaphore doesn't rotate into the placement waits on
                # sync; later blocks wait for the T8 replication copies
                if ib == 0:
                    mdma = nc.scalar.dma_start(mk[:], m_in[0:128, :])
                    add_dep_helper(mdma.ins, zcol_p.ins,
                                   reason="defer mask stream behind prep")
                    add_dep_helper(mdma.ins, zrow_p.ins,
                                   reason="defer mask stream behind prep")
                else:
                    mdma = nc.sync.dma_start(mk[:],
                                             m_in[ib * 128:(ib + 1) * 128, :])
                    add_dep_helper(mdma.ins, zcol_tail.ins,
                                   reason="defer mask stream behind prep")
                    add_dep_helper(mdma.ins, zrow_tail.ins,
                                   reason="defer mask stream behind prep")
                # exp results for a whole row block
                eb = expp.tile([128, n], bf16)
                # chunking: fine at the start to fill the pipeline fast
                if ib == 0:
                    awidths, swidths = [512] * 4, [512] * 4
                elif ib == 1:
                    awidths, swidths = [1024] * 2, [1024] * 2
                else:
                    awidths, swidths = [2048], [2048]
                ps = psump.tile([128, n], f32)
                col = 0
                for wch in awidths:
                    for jc in range(wch // 512):
                        c0 = col + jc * 512
                        g = 64 * ((c0 // 512) % 2)  # PE sub-tile T0 / T8
                        nc.tensor.matmul(
                            ps[:, c0:c0 + 512],
                            zrow[g:g + 14, ib * 128:(ib + 1) * 128],
                            zcol[g:g + 14, c0:c0 + 512],
                            start=True,
                            stop=True,
                        )
                    nc.scalar.activation(eb[:, col:col + wch],
                                         ps[:, col:col + wch], AF.Exp,
                                         scale=-0.1)
                    col += wch
                col = 0
                for wch in swidths:
                    nc.vector.scalar_tensor_tensor(
                        eb[:, col:col + wch], eb[:, col:col + wch], 1.0,
                        mk[:, col:col + wch],
                        op0=ALU.subtract, op1=ALU.mult,
                        accum_out=acc[:, acol:acol + 1],
                    )
                    col += wch
                    acol += 1
            # [128, acol] partials -> one on-device scalar (a [1, 1] DMA
            # needs a single descriptor; [128, 1] needs 128)
            accr = singles.tile([128, 1], f32)
            nc.vector.tensor_reduce(accr[:], acc[:, 0:acol],
                                    mybir.AxisListType.X, ALU.add)
            accs = singles.tile([1, 1], f32)
            nc.gpsimd.tensor_reduce(accs[:], accr[:],
                                    mybir.AxisListType.C, ALU.add)
            nc.scalar.dma_start(out[:], accs[:])
    nc.compile()
    return nc


def _get_graph():
    global _CACHED
    if _CACHED is None:
        _CACHED = _build()
    return _CACHED


def _pack_ea(e, a, n, G=8):
    w = n // G
    ea = np.empty((2 * G, 2 * w), dtype=np.float32)
    for d in range(2):
        ea[d * G:(d + 1) * G, :w] = e[:, d].reshape(G, w)
        ea[d * G:(d + 1) * G, w:] = a[:, d].reshape(G, w)
    return ea


def kernel(embedding, abs_coords, patch_mask, _trace=False, _trace_kwargs=None):
    import ml_dtypes
    from concourse.bass_utils import run_bass_kernel_spmd

    nc = _get_graph()
    mask_bf16 = np.ascontiguousarray(patch_mask).astype(ml_dtypes.bfloat16)
    in_maps = [
        {
            "ea": _pack_ea(embedding[b], abs_coords[b], N),
            "m": mask_bf16[b],
        }
        for b in range(B)
    ]
    kw = {}
    if _trace:
        kw = dict(trace=True, **(_trace_kwargs or {}))
    res = None
    last_err = None
    for _attempt in range(3):
        try:
            res = run_bass_kernel_spmd(nc, in_maps, core_ids=list(range(B)), **kw)
            # force materialization so device-side failures surface here
            total = -sum(
                float(np.sum(np.asarray(r["out"]), dtype=np.float64))
                for r in res.results
            )
            break
        except Exception as err:  # transient device faults: retry
            last_err = err
            res = None
    if res is None:
        raise last_err
    out = np.float32(total)
    if _trace:
        return out, res
    return out
